# revision 1
# baseline (speedup 1.0000x reference)
"""Multi-head causal attention (B=2, T=2048, D=1024, H=16, Hd=64) on 8 trn2 cores.

Sharding: batch x head-group. Core c handles batch c//4 and heads
(c%4)*4 .. (c%4)*4+3 (data + tensor/head parallel). Each core computes
Q/K/V projections for its 4 heads, causal attention, and a partial
output projection (row-slice of Wo); the host sums the 4 partials per
batch and adds bo.

Device layout notes:
- Host passes x^T (q/k/v transposed to [D, T]) in bf16 so every matmul
  contraction has its operand partition-major; no on-chip transposes.
- Scores are computed transposed (S^T[t2, t1] = K^T.T @ Q^T) so softmax
  sums land on the PE via an appended ones-column in V (row 64 of the
  O^T psum accumulates the denominators for free).
- No max-subtraction in softmax: scaled scores are bounded (|S|/8 < 9
  for N(0,1)-scale inputs; exp stays far from fp32 overflow).
- Normalization: denominator row -> fp32 outer-product broadcast on PE
  -> DVE reciprocal -> DVE multiply into O^T (bf16).
"""

import os
import sys

for _p in ("/root/.axon_site/_ro/trn_rl_repo", "/opt/trn_rl_repo"):
    if _p not in sys.path and os.path.isdir(_p):
        sys.path.append(_p)

import numpy as np
import ml_dtypes

B, T, D = 2, 2048, 1024
H, HD = 16, 64
HPC = 4                # heads per core
DH = HPC * HD          # 256 head-dim cols per core
KC = D // 128          # 8 contraction chunks
NT4 = T // 512         # 4 t1-chunks
NB = T // 128          # 16 t2-blocks
N_CORES = 8

_BF16 = ml_dtypes.bfloat16
_cache = {}


def _build():
    import concourse.bass as bass
    import concourse.tile as tile
    from concourse import bacc, mybir

    f32 = mybir.dt.float32
    bf16 = mybir.dt.bfloat16
    Exp = mybir.ActivationFunctionType.Exp
    Identity = mybir.ActivationFunctionType.Identity

    nc = bacc.Bacc(target_bir_lowering=False)

    xqt_d = nc.declare_dram_parameter("xqt", [D, T], bf16, isOutput=False)
    xkt_d = nc.declare_dram_parameter("xkt", [D, T], bf16, isOutput=False)
    xvt_d = nc.declare_dram_parameter("xvt", [D, T], bf16, isOutput=False)
    wq_d = nc.declare_dram_parameter("wq", [D, DH], bf16, isOutput=False)
    wk_d = nc.declare_dram_parameter("wk", [D, DH], bf16, isOutput=False)
    wv_d = nc.declare_dram_parameter("wv", [D, DH], bf16, isOutput=False)
    wo_d = nc.declare_dram_parameter("wo", [DH, D], bf16, isOutput=False)
    bq_d = nc.declare_dram_parameter("bq2", [2, 128], f32, isOutput=False)
    bk_d = nc.declare_dram_parameter("bk2", [2, 128], f32, isOutput=False)
    bv_d = nc.declare_dram_parameter("bv1", [1, DH], bf16, isOutput=False)
    tri_d = nc.declare_dram_parameter("tri", [128, 128], bf16, isOutput=False)
    out_d = nc.declare_dram_parameter("out", [T, D], f32, isOutput=True)

    with tile.TileContext(nc) as tc:
        with tc.tile_pool(name="const", bufs=1) as const, \
             tc.tile_pool(name="xpool", bufs=10) as xpool, \
             tc.tile_pool(name="ptp", bufs=12) as ptp, \
             tc.tile_pool(name="bcp", bufs=2) as bcp, \
             tc.tile_pool(name="outp", bufs=2) as outp, \
             tc.tile_pool(name="ps_a", bufs=2, space="PSUM") as ps_a, \
             tc.tile_pool(name="ps_o", bufs=1, space="PSUM") as ps_o, \
             tc.tile_pool(name="ps_v", bufs=2, space="PSUM") as ps_v:

            # ---- constants ----
            wq_sb = const.tile([128, KC, DH], bf16)
            wk_sb = const.tile([128, KC, DH], bf16)
            wv_sb = const.tile([128, KC, DH], bf16)
            nc.sync.dma_start(out=wq_sb[:], in_=wq_d[:].rearrange("(k p) n -> p k n", p=128))
            nc.sync.dma_start(out=wk_sb[:], in_=wk_d[:].rearrange("(k p) n -> p k n", p=128))
            nc.sync.dma_start(out=wv_sb[:], in_=wv_d[:].rearrange("(k p) n -> p k n", p=128))
            wo_sb = const.tile([128, 2, D], bf16)
            nc.sync.dma_start(out=wo_sb[:], in_=wo_d[:].rearrange("(c p) n -> p c n", p=128))
            bq_sb = const.tile([128, 2], f32)
            bk_sb = const.tile([128, 2], f32)
            for c in range(2):
                nc.sync.dma_start(out=bq_sb[:, c : c + 1], in_=bq_d[c, :].unsqueeze(1))
                nc.sync.dma_start(out=bk_sb[:, c : c + 1], in_=bk_d[c, :].unsqueeze(1))
            tri_sb = const.tile([128, 128], bf16)
            nc.sync.dma_start(out=tri_sb[:], in_=tri_d[:])
            bv_sb = const.tile([1, DH], bf16)
            nc.sync.dma_start(out=bv_sb[:], in_=bv_d[:])
            ones_bf = const.tile([1, 128], bf16)
            nc.vector.memset(ones_bf[:], 1.0)
            ones_f32 = const.tile([1, 128], f32)
            nc.vector.memset(ones_f32[:], 1.0)

            # bv broadcast tile [128, DH]; the redundant overwrites double as
            # PE warm-up (HAM un-throttle) while the x^T DMA loads stream in.
            bvb_ps = ps_v.tile([128, DH], f32, tag="vps")
            for _ in range(24):
                nc.tensor.matmul(bvb_ps[:], ones_bf[:], bv_sb[:], start=True, stop=True)
            bvb_sb = const.tile([128, DH], bf16)
            nc.vector.tensor_copy(out=bvb_sb[:], in_=bvb_ps[:])

            # ---- persistent activations ----
            qt_sb = [const.tile([128, T], bf16, tag=f"qt{i}", name=f"qt{i}") for i in range(2)]
            kt_sb = [const.tile([128, T], bf16, tag=f"kt{i}", name=f"kt{i}") for i in range(2)]
            ont_sb = [const.tile([128, T], bf16, tag=f"ont{i}", name=f"ont{i}") for i in range(2)]
            vaug_sb = const.tile([128, NB, HPC * (HD + 1)], bf16)
            # ones columns for the denominator trick
            nc.vector.memset(
                vaug_sb[:].rearrange("p b (h x) -> p b h x", h=HPC)[:, :, :, HD : HD + 1],
                1.0,
            )

            # ---- phase 1: Q^T / K^T projections ----
            # Q^T[dh, t] accumulated over k: lhsT = W chunk [128, 128], rhs = x^T chunk [128, 512]
            for which, (xt_d, w_sb, b_sb, dst) in enumerate(
                [(xqt_d, wq_sb, bq_sb, qt_sb), (xkt_d, wk_sb, bk_sb, kt_sb)]
            ):
                xch = []
                for k in range(KC):
                    xt = xpool.tile([128, T], bf16, tag="x")
                    nc.sync.dma_start(out=xt[:], in_=xt_d[k * 128 : (k + 1) * 128, :])
                    xch.append(xt)
                for dhc in range(2):
                    for t4 in range(NT4):
                        ps = ps_a.tile([128, 512], f32, tag="sa")
                        for k in range(KC):
                            nc.tensor.matmul(
                                ps[:],
                                w_sb[:, k, dhc * 128 : (dhc + 1) * 128],
                                xch[k][:, t4 * 512 : (t4 + 1) * 512],
                                start=(k == 0),
                                stop=(k == KC - 1),
                            )
                        nc.scalar.activation(
                            out=dst[dhc][:, t4 * 512 : (t4 + 1) * 512],
                            in_=ps[:],
                            func=Identity,
                            bias=b_sb[:, dhc : dhc + 1],
                            scale=1.0,
                        )

            # xv chunks stay resident for all V-block projections
            xvch = []
            for k in range(KC):
                xt = xpool.tile([128, T], bf16, tag="x")
                nc.sync.dma_start(out=xt[:], in_=xvt_d[k * 128 : (k + 1) * 128, :])
                xvch.append(xt)

            # ---- phases 2+3: attention with fine-grained interleave ----
            # S tiles are emitted in 2-block pairs sharing one 2-bank psum
            # tile so full pairs need a single (cheaper) exp op. PV(h) and
            # S(h+1) alternate so the PE always has independent work while
            # ACT drains exps; V-projection and output-projection units drip
            # into the stream as PE fillers. Output projection writes its
            # psum straight to DRAM via DMA (no DVE eviction).
            def make_v_unit(tb):
                def emit():
                    ps = ps_v.tile([128, DH], f32, tag="vps", name="v_ps")
                    for k in range(KC):
                        nc.tensor.matmul(
                            ps[:],
                            xvch[k][:, tb * 128 : (tb + 1) * 128],
                            wv_sb[:, k, :],
                            start=(k == 0),
                            stop=(k == KC - 1),
                        )
                    nc.vector.tensor_add(
                        vaug_sb[:, tb, :].rearrange("p (h x) -> p h x", h=HPC)[:, :, 0:HD],
                        ps[:].rearrange("p (h x) -> p h x", h=HPC),
                        bvb_sb[:].rearrange("p (h x) -> p h x", h=HPC),
                    )
                return emit

            def make_outproj_unit(m):
                def emit():
                    ps = ps_a.tile([128, 2, 512], f32, tag="sa", name="op_ps")
                    ob = outp.tile([128, D], f32, tag="ob", name="ob")
                    for n2 in range(2):
                        for dhc in range(2):
                            nc.tensor.matmul(
                                ps[:, n2, :],
                                ont_sb[dhc][:, m * 128 : (m + 1) * 128],
                                wo_sb[:, dhc, n2 * 512 : (n2 + 1) * 512],
                                start=(dhc == 0),
                                stop=(dhc == 1),
                            )
                        nc.vector.tensor_copy(
                            out=ob[:, n2 * 512 : (n2 + 1) * 512], in_=ps[:, n2, :]
                        )
                    nc.sync.dma_start(out=out_d[m * 128 : (m + 1) * 128, :], in_=ob[:])
                return emit

            for c in range(NT4):
                nblk = 4 * c + 4

                def s_pair(h, bp):
                    # blocks b0=2bp, b1=2bp+1 share one [128, 2, 512] psum tile
                    hc, hr = h // 2, (h % 2) * 64
                    s_ps = ps_a.tile([128, 2, 512], f32, tag="sa", name="s_ps")
                    pt = ptp.tile([128, 2, 512], bf16, tag="pt", name="pt")
                    geo = []
                    for i in range(2):
                        b = 2 * bp + i
                        r = b - 4 * c
                        off = max(r, 0) * 128
                        w = 512 - off
                        geo.append((b, r, off, w))
                        nc.tensor.matmul(
                            s_ps[:, i, off : off + w],
                            kt_sb[hc][hr : hr + 64, b * 128 : (b + 1) * 128],
                            qt_sb[hc][hr : hr + 64, c * 512 + off : (c + 1) * 512],
                            start=True,
                            stop=True,
                        )
                    if geo[0][1] < 0 and geo[1][1] < 0:
                        # both below the diagonal: one merged exp over 1024 cols
                        nc.scalar.activation(
                            out=pt[:], in_=s_ps[:], func=Exp, scale=0.125
                        )
                    else:
                        for i, (b, r, off, w) in enumerate(geo):
                            nc.scalar.activation(
                                out=pt[:, i, off : off + w],
                                in_=s_ps[:, i, off : off + w],
                                func=Exp,
                                scale=0.125,
                            )
                    for i, (b, r, off, w) in enumerate(geo):
                        if r >= 0:
                            nc.vector.tensor_mul(
                                pt[:, i, off : off + 128],
                                pt[:, i, off : off + 128],
                                tri_sb[:],
                            )
                    return (pt, geo)

                def pv_block(h, b, pairs, o_ps):
                    pt, geo = pairs[b // 2]
                    i = b % 2
                    _, r, off, w = geo[i]
                    nc.tensor.matmul(
                        o_ps[:, off : off + w],
                        vaug_sb[:, b, h * (HD + 1) : (h + 1) * (HD + 1)],
                        pt[:, i, off : off + w],
                        start=(b == 0),
                        stop=(b == nblk - 1),
                    )

                def norm(h, o_ps):
                    hc, hr = h // 2, (h % 2) * 64
                    den_f = bcp.tile([1, 512], f32, tag="den", name="den_f")
                    nc.vector.tensor_copy(out=den_f[:], in_=o_ps[64 : HD + 1, :])
                    bc_sb = bcp.tile([64, 512], f32, tag="bcs", name="bc_sb", bufs=2)
                    nc.gpsimd.partition_broadcast(bc_sb[:], den_f[:])
                    bcb = bcp.tile([64, 512], f32, tag="bcb", name="bcb", bufs=2)
                    nc.vector.reciprocal_approx_fast(out=bcb[:], in_=bc_sb[:])
                    nc.vector.tensor_mul(
                        ont_sb[hc][hr : hr + 64, c * 512 : (c + 1) * 512],
                        o_ps[0:HD, :],
                        bcb[:],
                    )

                v_units = [make_v_unit(tb) for tb in range(4 * c, 4 * c + 4)]
                op_units = (
                    [make_outproj_unit(m) for m in range(4 * (c - 1), 4 * c)]
                    if c > 0
                    else []
                )

                o_pss = [
                    ps_o.tile([HD + 1, 512], f32, tag=f"ops{h % 2}", name=f"ops{h}")
                    for h in range(HPC)
                ]
                npair = nblk // 2
                ptss = {}

                # stream A: scores(0) pairs with V units interleaved
                ptss[0] = []
                for bp in range(npair):
                    ptss[0].append(s_pair(0, bp))
                    if v_units:
                        v_units.pop(0)()
                while v_units:
                    v_units.pop(0)()

                # streams B-E: S(h+1) pairs and PV(h) alternate; outproj drips
                for h in range(HPC):
                    hn = h + 1
                    if hn < HPC:
                        ptss[hn] = []
                    for bp in range(npair):
                        if hn < HPC:
                            ptss[hn].append(s_pair(hn, bp))
                        pv_block(h, 2 * bp, ptss[h], o_pss[h])
                        pv_block(h, 2 * bp + 1, ptss[h], o_pss[h])
                        if op_units and bp % 2 == 1:
                            op_units.pop(0)()
                    ptss.pop(h)
                    norm(h, o_pss[h])
                while op_units:
                    op_units.pop(0)()

            # final chunk's output projection
            for m in range(4 * (NT4 - 1), 4 * NT4):
                make_outproj_unit(m)()

    nc.compile()
    return nc


def _get_nc():
    if "nc" not in _cache:
        _cache["nc"] = _build()
    return _cache["nc"]


def build_in_maps(query, key, value, Wq, bq, Wk, bk, Wv, bv, Wo, bo):
    query = np.asarray(query, np.float32)
    key = np.asarray(key, np.float32)
    value = np.asarray(value, np.float32)
    Wq_, Wk_, Wv_, Wo_ = (np.asarray(a, np.float32) for a in (Wq, Wk, Wv, Wo))
    bq_, bk_, bv_, bo_ = (np.asarray(a, np.float32) for a in (bq, bk, bv, bo))

    xqt = [np.ascontiguousarray(query[b].T).astype(_BF16) for b in range(B)]
    xkt = [np.ascontiguousarray(key[b].T).astype(_BF16) for b in range(B)]
    xvt = [np.ascontiguousarray(value[b].T).astype(_BF16) for b in range(B)]

    tri = np.tril(np.ones((128, 128), np.float32)).T.astype(_BF16)  # tri[j,i]=1 iff j<=i

    in_maps = []
    for c in range(N_CORES):
        b, hg = c // 4, c % 4
        sl = slice(hg * DH, (hg + 1) * DH)
        in_maps.append(
            {
                "xqt": xqt[b],
                "xkt": xkt[b],
                "xvt": xvt[b],
                "wq": np.ascontiguousarray(Wq_[:, sl]).astype(_BF16),
                "wk": np.ascontiguousarray(Wk_[:, sl]).astype(_BF16),
                "wv": np.ascontiguousarray(Wv_[:, sl]).astype(_BF16),
                "wo": np.ascontiguousarray(Wo_[sl, :]).astype(_BF16),
                "bq2": np.ascontiguousarray(bq_[sl].reshape(2, 128)),
                "bk2": np.ascontiguousarray(bk_[sl].reshape(2, 128)),
                "bv1": bv_[sl].reshape(1, DH).astype(_BF16),
                "tri": tri,
            }
        )

    return in_maps, bo_


def kernel(query, key, value, Wq, bq, Wk, bk, Wv, bv, Wo, bo):
    from concourse.bass_utils import run_bass_kernel_spmd

    nc = _get_nc()
    in_maps, bo_ = build_in_maps(query, key, value, Wq, bq, Wk, bk, Wv, bv, Wo, bo)
    res = run_bass_kernel_spmd(nc, in_maps, list(range(N_CORES)))
    _cache["last_results"] = res

    out = np.empty((B, T, D), np.float32)
    for b in range(B):
        acc = res.results[4 * b]["out"].astype(np.float32).copy()
        for hg in range(1, 4):
            acc += res.results[4 * b + hg]["out"]
        out[b] = acc + bo_[None, :]
    return out



# revision 3
# speedup vs baseline: 1.1417x; 1.1417x over previous
"""Multi-head causal attention (B=2, T=2048, D=1024, H=16, Hd=64) on 8 trn2 cores.

Sharding: batch x head-group. Core c handles batch c//4 and heads
(c%4)*4 .. (c%4)*4+3 (data + tensor/head parallel). Each core computes
Q/K/V projections for its 4 heads, causal attention, and a partial
output projection (row-slice of Wo); the host sums the 4 partials per
batch and adds bo.

Device layout notes:
- Host passes x^T (q/k/v transposed to [D, T]) in bf16 so every matmul
  contraction has its operand partition-major; no on-chip transposes.
- Scores are computed transposed (S^T[t2, t1] = K^T.T @ Q^T) so softmax
  sums land on the PE via an appended ones-column in V (row 64 of the
  O^T psum accumulates the denominators for free).
- No max-subtraction in softmax: scaled scores are bounded (|S|/8 < 9
  for N(0,1)-scale inputs; exp stays far from fp32 overflow).
- Normalization: denominator row -> fp32 outer-product broadcast on PE
  -> DVE reciprocal -> DVE multiply into O^T (bf16).
- Q/K projections run k-outer so the PE rides the x^T DMA stream; x
  arrives in 4 double-chunk tiles per tensor (fewer, larger DMAs keep
  the Sync issue queue off the critical path).
- Output partials leave in fp16 (halves the 8MB/core writeback), two
  row-blocks per DMA.
"""

import os
import sys

for _p in ("/root/.axon_site/_ro/trn_rl_repo", "/opt/trn_rl_repo"):
    if _p not in sys.path and os.path.isdir(_p):
        sys.path.append(_p)

import numpy as np
import ml_dtypes

B, T, D = 2, 2048, 1024
H, HD = 16, 64
HPC = 4                # heads per core
DH = HPC * HD          # 256 head-dim cols per core
KC = D // 128          # 8 contraction chunks
KC2 = KC // 2          # 4 double-chunk x tiles
NT4 = T // 512         # 4 t1-chunks
NB = T // 128          # 16 t2-blocks
N_CORES = 8

_BF16 = ml_dtypes.bfloat16
_F16 = np.float16
_cache = {}


def _build():
    import concourse.bass as bass
    import concourse.tile as tile
    from concourse import bacc, mybir

    f32 = mybir.dt.float32
    f16 = mybir.dt.float16
    bf16 = mybir.dt.bfloat16
    Exp = mybir.ActivationFunctionType.Exp
    Identity = mybir.ActivationFunctionType.Identity

    nc = bacc.Bacc(target_bir_lowering=False)

    xqt_d = nc.declare_dram_parameter("xqt", [D, T], bf16, isOutput=False)
    xkt_d = nc.declare_dram_parameter("xkt", [D, T], bf16, isOutput=False)
    xvt_d = nc.declare_dram_parameter("xvt", [D, T], bf16, isOutput=False)
    wq_d = nc.declare_dram_parameter("wq", [D, DH], bf16, isOutput=False)
    wk_d = nc.declare_dram_parameter("wk", [D, DH], bf16, isOutput=False)
    wv_d = nc.declare_dram_parameter("wv", [D, DH], bf16, isOutput=False)
    wo_d = nc.declare_dram_parameter("wo", [DH, D], bf16, isOutput=False)
    bq_d = nc.declare_dram_parameter("bq2", [128, 2], f32, isOutput=False)
    bk_d = nc.declare_dram_parameter("bk2", [128, 2], f32, isOutput=False)
    bv_d = nc.declare_dram_parameter("bv1", [1, DH], bf16, isOutput=False)
    tri_d = nc.declare_dram_parameter("tri", [128, 128], bf16, isOutput=False)
    out_d = nc.declare_dram_parameter("out", [T, D], f16, isOutput=True)

    with tile.TileContext(nc) as tc:
        with tc.tile_pool(name="const", bufs=1) as const, \
             tc.tile_pool(name="xpool", bufs=12) as xpool, \
             tc.tile_pool(name="ptp", bufs=12) as ptp, \
             tc.tile_pool(name="bcp", bufs=2) as bcp, \
             tc.tile_pool(name="outp", bufs=2) as outp, \
             tc.tile_pool(name="ps_a", bufs=2, space="PSUM") as ps_a, \
             tc.tile_pool(name="ps_o", bufs=1, space="PSUM") as ps_o, \
             tc.tile_pool(name="ps_v", bufs=2, space="PSUM") as ps_v:

            # ---- small constants first (cheap DMAs, unblock warm-up) ----
            bv_sb = const.tile([1, DH], bf16)
            nc.sync.dma_start(out=bv_sb[:], in_=bv_d[:])
            bq_sb = const.tile([128, 2], f32)
            bk_sb = const.tile([128, 2], f32)
            nc.sync.dma_start(out=bq_sb[:], in_=bq_d[:])
            nc.sync.dma_start(out=bk_sb[:], in_=bk_d[:])
            tri_sb = const.tile([128, 128], bf16)
            nc.sync.dma_start(out=tri_sb[:], in_=tri_d[:])
            ones_bf = const.tile([1, 128], bf16)
            nc.vector.memset(ones_bf[:], 1.0)
            ones_f32 = const.tile([1, 128], f32)
            nc.vector.memset(ones_f32[:], 1.0)

            # wq + xq stream ahead of everything else the PE needs first
            wq_sb = const.tile([128, KC, DH], bf16)
            nc.sync.dma_start(out=wq_sb[:], in_=wq_d[:].rearrange("(k p) n -> p k n", p=128))

            xqch = []
            for k2 in range(KC2):
                xt = xpool.tile([128, 2, T], bf16, tag="x")
                nc.sync.dma_start(
                    out=xt[:],
                    in_=xqt_d[k2 * 256 : (k2 + 1) * 256, :].rearrange(
                        "(k p) n -> p k n", p=128
                    ),
                )
                xqch.append(xt)

            wk_sb = const.tile([128, KC, DH], bf16)
            nc.sync.dma_start(out=wk_sb[:], in_=wk_d[:].rearrange("(k p) n -> p k n", p=128))
            xkch = []
            for k2 in range(KC2):
                xt = xpool.tile([128, 2, T], bf16, tag="x")
                nc.sync.dma_start(
                    out=xt[:],
                    in_=xkt_d[k2 * 256 : (k2 + 1) * 256, :].rearrange(
                        "(k p) n -> p k n", p=128
                    ),
                )
                xkch.append(xt)

            wv_sb = const.tile([128, KC, DH], bf16)
            nc.sync.dma_start(out=wv_sb[:], in_=wv_d[:].rearrange("(k p) n -> p k n", p=128))
            xvch2 = []
            for k2 in range(KC2):
                xt = xpool.tile([128, 2, T], bf16, tag="x")
                nc.sync.dma_start(
                    out=xt[:],
                    in_=xvt_d[k2 * 256 : (k2 + 1) * 256, :].rearrange(
                        "(k p) n -> p k n", p=128
                    ),
                )
                xvch2.append(xt)
            wo_sb = const.tile([128, 2, D], bf16)
            nc.sync.dma_start(out=wo_sb[:], in_=wo_d[:].rearrange("(c p) n -> p c n", p=128))

            def xq(k):
                return xqch[k // 2][:, k % 2, :]

            def xk(k):
                return xkch[k // 2][:, k % 2, :]

            def xv(k):
                return xvch2[k // 2][:, k % 2, :]

            # bv broadcast tile [128, DH]; the redundant overwrites double as
            # PE warm-up (HAM un-throttle) while the x^T DMA loads stream in.
            bvb_ps = ps_v.tile([128, DH], f32, tag="vps")
            for _ in range(24):
                nc.tensor.matmul(bvb_ps[:], ones_bf[:], bv_sb[:], start=True, stop=True)
            bvb_sb = const.tile([128, DH], bf16)
            nc.vector.tensor_copy(out=bvb_sb[:], in_=bvb_ps[:])

            # ---- persistent activations ----
            qt_sb = [const.tile([128, T], bf16, tag=f"qt{i}", name=f"qt{i}") for i in range(2)]
            kt_sb = [const.tile([128, T], bf16, tag=f"kt{i}", name=f"kt{i}") for i in range(2)]
            ont_sb = [const.tile([128, T], bf16, tag=f"ont{i}", name=f"ont{i}") for i in range(2)]
            vaug_sb = const.tile([128, NB, HPC * (HD + 1)], bf16)
            # ones columns for the denominator trick
            nc.vector.memset(
                vaug_sb[:].rearrange("p b (h x) -> p b h x", h=HPC)[:, :, :, HD : HD + 1],
                1.0,
            )

            # ---- phase 1: Q^T / K^T projections, k-outer so the PE starts
            # on chunk 0 while later chunks are still in flight ----
            for which, (xf, w_sb, b_sb, dst) in enumerate(
                [(xq, wq_sb, bq_sb, qt_sb), (xk, wk_sb, bk_sb, kt_sb)]
            ):
                for dhc in range(2):
                    psA = ps_a.tile([128, 2, 512], f32, tag="sa")
                    psB = ps_a.tile([128, 2, 512], f32, tag="sa")
                    for k in range(KC):
                        w = w_sb[:, k, dhc * 128 : (dhc + 1) * 128]
                        for t4, ps in ((0, psA), (1, psA), (2, psB), (3, psB)):
                            nc.tensor.matmul(
                                ps[:, t4 % 2, :],
                                w,
                                xf(k)[:, t4 * 512 : (t4 + 1) * 512],
                                start=(k == 0),
                                stop=(k == KC - 1),
                            )
                    for half, ps in ((0, psA), (1, psB)):
                        nc.scalar.activation(
                            out=dst[dhc][:, half * 1024 : (half + 1) * 1024],
                            in_=ps[:].rearrange("p a n -> p (a n)"),
                            func=Identity,
                            bias=b_sb[:, dhc : dhc + 1],
                            scale=1.0,
                        )

            # ---- phases 2+3: attention with fine-grained interleave ----
            # S tiles are emitted in 2-block pairs sharing one 2-bank psum
            # tile so full pairs need a single (cheaper) exp op. PV(h) and
            # S(h+1) alternate so the PE always has independent work while
            # ACT drains exps; V-projection and output-projection units drip
            # into the stream as PE fillers.
            def make_v_unit(tb):
                def emit():
                    ps = ps_v.tile([128, DH], f32, tag="vps", name="v_ps")
                    for k in range(KC):
                        nc.tensor.matmul(
                            ps[:],
                            xv(k)[:, tb * 128 : (tb + 1) * 128],
                            wv_sb[:, k, :],
                            start=(k == 0),
                            stop=(k == KC - 1),
                        )
                    nc.vector.tensor_add(
                        vaug_sb[:, tb, :].rearrange("p (h x) -> p h x", h=HPC)[:, :, 0:HD],
                        ps[:].rearrange("p (h x) -> p h x", h=HPC),
                        bvb_sb[:].rearrange("p (h x) -> p h x", h=HPC),
                    )
                return emit

            # output projection: two row-blocks share an outp tile; the DMA
            # fires on the odd block (2 blocks per transfer).
            ob_tiles = {}

            def make_outproj_unit(m):
                def emit():
                    ps = ps_a.tile([128, 2, 512], f32, tag="sa", name="op_ps")
                    if m % 2 == 0:
                        ob_tiles[m // 2] = outp.tile([128, 2, D], f16, tag="ob", name="ob")
                    ob = ob_tiles[m // 2]
                    for n2 in range(2):
                        for dhc in range(2):
                            nc.tensor.matmul(
                                ps[:, n2, :],
                                ont_sb[dhc][:, m * 128 : (m + 1) * 128],
                                wo_sb[:, dhc, n2 * 512 : (n2 + 1) * 512],
                                start=(dhc == 0),
                                stop=(dhc == 1),
                            )
                        nc.vector.tensor_copy(
                            out=ob[:, m % 2, n2 * 512 : (n2 + 1) * 512],
                            in_=ps[:, n2, :],
                        )
                    if m % 2 == 1:
                        nc.sync.dma_start(
                            out=out_d[(m - 1) * 128 : (m + 1) * 128, :].rearrange(
                                "(c p) n -> p c n", p=128
                            ),
                            in_=ob[:],
                        )
                return emit

            for c in range(NT4):
                nblk = 4 * c + 4

                def s_pair(h, bp):
                    # blocks b0=2bp, b1=2bp+1 share one [128, 2, 512] psum tile
                    hc, hr = h // 2, (h % 2) * 64
                    s_ps = ps_a.tile([128, 2, 512], f32, tag="sa", name="s_ps")
                    pt = ptp.tile([128, 2, 512], bf16, tag="pt", name="pt")
                    geo = []
                    for i in range(2):
                        b = 2 * bp + i
                        r = b - 4 * c
                        off = max(r, 0) * 128
                        w = 512 - off
                        geo.append((b, r, off, w))
                        nc.tensor.matmul(
                            s_ps[:, i, off : off + w],
                            kt_sb[hc][hr : hr + 64, b * 128 : (b + 1) * 128],
                            qt_sb[hc][hr : hr + 64, c * 512 + off : (c + 1) * 512],
                            start=True,
                            stop=True,
                        )
                    if geo[0][1] < 0 and geo[1][1] < 0:
                        # both below the diagonal: one merged exp over 1024 cols
                        nc.scalar.activation(
                            out=pt[:], in_=s_ps[:], func=Exp, scale=0.125
                        )
                    else:
                        for i, (b, r, off, w) in enumerate(geo):
                            nc.scalar.activation(
                                out=pt[:, i, off : off + w],
                                in_=s_ps[:, i, off : off + w],
                                func=Exp,
                                scale=0.125,
                            )
                    for i, (b, r, off, w) in enumerate(geo):
                        if r >= 0:
                            nc.vector.tensor_mul(
                                pt[:, i, off : off + 128],
                                pt[:, i, off : off + 128],
                                tri_sb[:],
                            )
                    return (pt, geo)

                def pv_block(h, b, pairs, o_ps):
                    pt, geo = pairs[b // 2]
                    i = b % 2
                    _, r, off, w = geo[i]
                    nc.tensor.matmul(
                        o_ps[:, off : off + w],
                        vaug_sb[:, b, h * (HD + 1) : (h + 1) * (HD + 1)],
                        pt[:, i, off : off + w],
                        start=(b == 0),
                        stop=(b == nblk - 1),
                    )

                def norm(h, o_ps):
                    hc, hr = h // 2, (h % 2) * 64
                    den_f = bcp.tile([1, 512], f32, tag="den", name="den_f")
                    nc.vector.tensor_copy(out=den_f[:], in_=o_ps[64 : HD + 1, :])
                    bc_sb = bcp.tile([64, 512], f32, tag="bcs", name="bc_sb", bufs=2)
                    nc.gpsimd.partition_broadcast(bc_sb[:], den_f[:])
                    bcb = bcp.tile([64, 512], f32, tag="bcb", name="bcb", bufs=2)
                    nc.vector.reciprocal_approx_fast(out=bcb[:], in_=bc_sb[:])
                    nc.vector.tensor_mul(
                        ont_sb[hc][hr : hr + 64, c * 512 : (c + 1) * 512],
                        o_ps[0:HD, :],
                        bcb[:],
                    )

                v_units = [make_v_unit(tb) for tb in range(4 * c, 4 * c + 4)]
                op_units = (
                    [make_outproj_unit(m) for m in range(4 * (c - 1), 4 * c)]
                    if c > 0
                    else []
                )

                o_pss = [
                    ps_o.tile([HD + 1, 512], f32, tag=f"ops{h % 2}", name=f"ops{h}")
                    for h in range(HPC)
                ]
                npair = nblk // 2
                ptss = {}

                # stream A: scores(0) pairs with V units interleaved
                ptss[0] = []
                for bp in range(npair):
                    ptss[0].append(s_pair(0, bp))
                    if v_units:
                        v_units.pop(0)()
                while v_units:
                    v_units.pop(0)()

                # streams B-E: S(h+1) pairs and PV(h) alternate; outproj drips
                for h in range(HPC):
                    hn = h + 1
                    if hn < HPC:
                        ptss[hn] = []
                    for bp in range(npair):
                        if hn < HPC:
                            ptss[hn].append(s_pair(hn, bp))
                        pv_block(h, 2 * bp, ptss[h], o_pss[h])
                        pv_block(h, 2 * bp + 1, ptss[h], o_pss[h])
                        if op_units and bp % 2 == 1:
                            op_units.pop(0)()
                    ptss.pop(h)
                    norm(h, o_pss[h])
                while op_units:
                    op_units.pop(0)()

            # final chunk's output projection
            for m in range(4 * (NT4 - 1), 4 * NT4):
                make_outproj_unit(m)()

    nc.compile()
    return nc


def _get_nc():
    if "nc" not in _cache:
        _cache["nc"] = _build()
    return _cache["nc"]


def build_in_maps(query, key, value, Wq, bq, Wk, bk, Wv, bv, Wo, bo):
    query = np.asarray(query, np.float32)
    key = np.asarray(key, np.float32)
    value = np.asarray(value, np.float32)
    Wq_, Wk_, Wv_, Wo_ = (np.asarray(a, np.float32) for a in (Wq, Wk, Wv, Wo))
    bq_, bk_, bv_, bo_ = (np.asarray(a, np.float32) for a in (bq, bk, bv, bo))

    xqt = [np.ascontiguousarray(query[b].T).astype(_BF16) for b in range(B)]
    xkt = [np.ascontiguousarray(key[b].T).astype(_BF16) for b in range(B)]
    xvt = [np.ascontiguousarray(value[b].T).astype(_BF16) for b in range(B)]

    tri = np.tril(np.ones((128, 128), np.float32)).T.astype(_BF16)  # tri[j,i]=1 iff j<=i

    in_maps = []
    for c in range(N_CORES):
        b, hg = c // 4, c % 4
        sl = slice(hg * DH, (hg + 1) * DH)
        in_maps.append(
            {
                "xqt": xqt[b],
                "xkt": xkt[b],
                "xvt": xvt[b],
                "wq": np.ascontiguousarray(Wq_[:, sl]).astype(_BF16),
                "wk": np.ascontiguousarray(Wk_[:, sl]).astype(_BF16),
                "wv": np.ascontiguousarray(Wv_[:, sl]).astype(_BF16),
                "wo": np.ascontiguousarray(Wo_[sl, :]).astype(_BF16),
                "bq2": np.ascontiguousarray(bq_[sl].reshape(2, 128).T),
                "bk2": np.ascontiguousarray(bk_[sl].reshape(2, 128).T),
                "bv1": bv_[sl].reshape(1, DH).astype(_BF16),
                "tri": tri,
            }
        )

    return in_maps, bo_


def kernel(query, key, value, Wq, bq, Wk, bk, Wv, bv, Wo, bo):
    from concourse.bass_utils import run_bass_kernel_spmd

    nc = _get_nc()
    in_maps, bo_ = build_in_maps(query, key, value, Wq, bq, Wk, bk, Wv, bv, Wo, bo)
    res = run_bass_kernel_spmd(nc, in_maps, list(range(N_CORES)))
    _cache["last_results"] = res

    out = np.empty((B, T, D), np.float32)
    for b in range(B):
        acc = res.results[4 * b]["out"].astype(np.float32)
        for hg in range(1, 4):
            acc += res.results[4 * b + hg]["out"].astype(np.float32)
        out[b] = acc + bo_[None, :]
    return out
